# revision 2
# baseline (speedup 1.0000x reference)
"""BEiT 4-stage forward on 8 TRN2 NeuronCores (Bass/Tile, SPMD).

Strategy: sequence-sharding. Tokens padded 1025 -> 1040 = 8*130; core c owns
tokens [130c, 130c+130). Per layer one AllGather moves each core's local K
(feature-major) and V (token-major, with interleaved ones-columns that make
the attnV matmul emit softmax denominators as a 65th output row). Activations
live feature-major [128, 6*130]; LN stats come from ones-vector matmuls; LN
gamma / QK_SCALE / residual scales are folded into weights on the host;
softmax uses no max-subtraction (|scores| < ~3, fp32 exp is safe; padded keys
carry -30000 bias so exp underflows to exactly 0). All matmuls bf16 with fp32
PSUM accumulation.

Self-contained: hardcodes shapes, imports only concourse + numpy + ml_dtypes.
"""
import numpy as np
import ml_dtypes

import concourse.bass as bass
import concourse.tile as tile
from concourse import bacc, mybir
from concourse import bass_utils

P = 128
T = 130          # tokens per core
NCORES = 8
NK = NCORES * T  # padded keys = 1040
NKT = 9          # key tiles: 8*128 + 16
F = 768
FT = 6
H = 12
FH = 64
MLPD = 3072
MT = 24
L = 12
N = 1025
EPS = 1e-6
QK_SCALE = FH ** -0.5
STAGE_ENDS = (2, 5, 8, 11)

F32 = mybir.dt.float32
BF = mybir.dt.bfloat16
AF = mybir.ActivationFunctionType
ALU = mybir.AluOpType
BF16NP = np.dtype(ml_dtypes.bfloat16)

AGE = FT * P * T + T * (H * 65)   # 201240 ag elements per rank
VOFF = FT * P * T                 # v offset inside a rank block


def _ap_view(t, offset, dims):
    return bass.AP(tensor=t.tensor, offset=t.offset + offset, ap=list(dims))


# ---------------------------------------------------------------- builder --
def _emit_layernorm(tc, x, ones, eps_t, work, smalls, psA, tag):
    nc = tc.nc
    x_bf = work.tile([P, FT * T], BF, tag=f"{tag}_xbf")
    nc.vector.tensor_copy(out=x_bf, in_=x)
    xsq = work.tile([P, FT * T], BF, tag=f"{tag}_xsq")
    nc.vector.tensor_mul(out=xsq, in0=x_bf, in1=x_bf)
    ps_s = psA.tile([1, T], F32, tag="mm")
    ps_s2 = psA.tile([1, T], F32, tag="mm")
    for ft in range(FT):
        nc.tensor.matmul(ps_s, ones, x_bf[:, ft * T:(ft + 1) * T],
                         start=(ft == 0), stop=(ft == FT - 1))
    for ft in range(FT):
        nc.tensor.matmul(ps_s2, ones, xsq[:, ft * T:(ft + 1) * T],
                         start=(ft == 0), stop=(ft == FT - 1))
    mu = smalls.tile([1, T], F32, tag=f"{tag}_mu")
    nc.scalar.mul(out=mu, in_=ps_s, mul=1.0 / F)
    e2 = smalls.tile([1, T], F32, tag=f"{tag}_e2")
    nc.scalar.mul(out=e2, in_=ps_s2, mul=1.0 / F)
    var = smalls.tile([1, T], F32, tag=f"{tag}_var")
    nc.vector.tensor_mul(out=var, in0=mu, in1=mu)
    nc.vector.tensor_sub(out=var, in0=e2, in1=var)
    std = smalls.tile([1, T], F32, tag=f"{tag}_std")
    nc.scalar.activation(out=std, in_=var, func=AF.Sqrt, bias=eps_t, scale=1.0)
    rstd = smalls.tile([1, T], F32, tag=f"{tag}_rstd")
    nc.vector.reciprocal(out=rstd, in_=std)
    mu_bf = smalls.tile([1, T], BF, tag=f"{tag}_mubf")
    nc.vector.tensor_copy(out=mu_bf, in_=mu)
    rstd_bf = smalls.tile([1, T], BF, tag=f"{tag}_rstdbf")
    nc.vector.tensor_copy(out=rstd_bf, in_=rstd)
    mu_b = work.tile([P, FT * T], BF, tag=f"{tag}_mub")
    rs_b = work.tile([P, FT * T], BF, tag=f"{tag}_rsb")
    for ft in range(FT):
        nc.gpsimd.partition_broadcast(mu_b[:, ft * T:(ft + 1) * T], mu_bf)
        nc.gpsimd.partition_broadcast(rs_b[:, ft * T:(ft + 1) * T], rstd_bf)
    xln = work.tile([P, FT * T], BF, tag=f"{tag}_xln")
    nc.vector.tensor_sub(out=xln, in0=x_bf, in1=mu_b)
    nc.vector.tensor_mul(out=xln, in0=xln, in1=rs_b)
    return xln


def build_nc(L_run=L, stage_ends=STAGE_ENDS):
    nc = bacc.Bacc("TRN2", target_bir_lowering=False, debug=False,
                   enable_asserts=True, num_devices=NCORES)
    d = {}
    d["x0"] = nc.dram_tensor("x0", [P, FT * T], F32, kind="ExternalInput").ap()
    d["wqk"] = nc.dram_tensor("wqk", [L_run, 12, P, FT, P], BF,
                              kind="ExternalInput").ap()
    d["cqk"] = nc.dram_tensor("cqk", [L_run, P, 12], F32,
                              kind="ExternalInput").ap()
    d["wv"] = nc.dram_tensor("wv", [L_run, FT, P, H * 65], BF,
                             kind="ExternalInput").ap()
    d["cv"] = nc.dram_tensor("cv", [L_run, 1, H * 65], F32,
                             kind="ExternalInput").ap()
    d["wproj"] = nc.dram_tensor("wproj", [L_run, FT, P, FT, P], BF,
                                kind="ExternalInput").ap()
    d["wfc1"] = nc.dram_tensor("wfc1", [L_run, MT, P, FT, P], BF,
                               kind="ExternalInput").ap()
    d["cfc1"] = nc.dram_tensor("cfc1", [L_run, P, MT], F32,
                               kind="ExternalInput").ap()
    d["wfc2"] = nc.dram_tensor("wfc2", [L_run, FT, P, MT, P], BF,
                               kind="ExternalInput").ap()
    d["biasT"] = nc.dram_tensor("biasT", [L_run, NKT, P, H, T], BF,
                                kind="ExternalInput").ap()
    n_stages = max(sum(1 for s in stage_ends if s < L_run), 1)
    outs_d = nc.dram_tensor("outs", [n_stages, P, FT * T], F32,
                            kind="ExternalOutput").ap()

    with tile.TileContext(nc) as tc:
        _emit_body(tc, d, outs_d, L_run, stage_ends)
    nc.compile()
    return nc


def _emit_body(tc, d, outs_d, L_run, stage_ends):
    nc = tc.nc
    import contextlib
    ctx = contextlib.ExitStack()
    with ctx:
        pers = ctx.enter_context(tc.tile_pool(name="pers", bufs=1))
        work = ctx.enter_context(tc.tile_pool(name="work", bufs=2))
        tmpp = ctx.enter_context(tc.tile_pool(name="tmpp", bufs=3))
        wgt = ctx.enter_context(tc.tile_pool(name="wgt", bufs=3))
        wgt2 = ctx.enter_context(tc.tile_pool(name="wgt2", bufs=2))
        smalls = ctx.enter_context(tc.tile_pool(name="smalls", bufs=4))
        psA = ctx.enter_context(tc.tile_pool(name="psA", bufs=4, space="PSUM"))
        dram = ctx.enter_context(tc.tile_pool(name="dram", bufs=2, space="DRAM"))

        x = pers.tile([P, FT * T], F32, tag="x")
        nc.sync.dma_start(out=x, in_=d["x0"])
        ones = pers.tile([P, 1], BF, tag="ones")
        nc.vector.memset(ones, 1.0)
        eps_t = pers.tile([1, 1], F32, tag="eps")
        nc.vector.memset(eps_t, EPS)

        si = 0
        for l in range(L_run):
            cqk_sb = smalls.tile([P, 12], F32, tag="cqk")
            nc.sync.dma_start(out=cqk_sb, in_=d["cqk"][l])
            cfc1_sb = smalls.tile([P, MT], F32, tag="cfc1")
            nc.sync.dma_start(out=cfc1_sb, in_=d["cfc1"][l])
            cv_b = work.tile([P, H * 65], F32, tag="cv_b")
            nc.sync.dma_start(
                out=cv_b,
                in_=_ap_view(d["cv"], l * H * 65, [[0, P], [1, H * 65]]))

            xln = _emit_layernorm(tc, x, ones, eps_t, work, smalls, psA, "ln1")

            # K (emitted first so the AllGather can start early), then V,
            # then the collective, then Q (overlaps the collective).
            ag_in = dram.tile([AGE], BF, tag="ag_in")
            ag_out = dram.tile([NCORES * AGE], BF, tag="ag_out",
                               addr_space="Shared")
            k_sb = work.tile([P, FT * T], BF, tag="k_sb")
            q_sb = work.tile([P, FT * T], BF, tag="q_sb")
            for ot in range(6, 12):
                w_t = wgt.tile([P, FT * P], BF, tag="w768")
                nc.sync.dma_start(out=w_t, in_=d["wqk"][l, ot])
                ps = psA.tile([P, T], F32, tag="mm")
                for kt in range(FT):
                    nc.tensor.matmul(ps, w_t[:, kt * P:(kt + 1) * P],
                                     xln[:, kt * T:(kt + 1) * T],
                                     start=(kt == 0), stop=(kt == FT - 1))
                sl = k_sb[:, (ot - 6) * T:(ot - 5) * T]
                nc.vector.tensor_scalar_add(out=sl, in0=ps,
                                            scalar1=cqk_sb[:, ot:ot + 1])
                nc.sync.dma_start(
                    out=_ap_view(ag_in, (ot - 6) * P * T, [[T, P], [1, T]]),
                    in_=sl)
            wv_t = pers.tile([P, FT * H * 65], BF, tag="wv_t")
            for kt in range(FT):
                nc.sync.dma_start(out=wv_t[:, kt * H * 65:(kt + 1) * H * 65],
                                  in_=d["wv"][l, kt])
            for mt in range(2):
                rows = P if mt == 0 else T - P
                v_loc = work.tile([P, H * 65], BF, tag=f"v_loc{mt}")
                for nf in range(2):
                    ps = psA.tile([P, 390], F32, tag="mm")
                    for kt in range(FT):
                        nc.tensor.matmul(
                            ps[:rows, :],
                            xln[:, kt * T + mt * P:kt * T + mt * P + rows],
                            wv_t[:, kt * H * 65 + nf * 390:
                                 kt * H * 65 + (nf + 1) * 390],
                            start=(kt == 0), stop=(kt == FT - 1))
                    nc.vector.tensor_add(
                        out=v_loc[:rows, nf * 390:(nf + 1) * 390],
                        in0=ps[:rows, :],
                        in1=cv_b[:rows, nf * 390:(nf + 1) * 390])
                nc.sync.dma_start(
                    out=_ap_view(ag_in, VOFF + mt * P * (H * 65),
                                 [[H * 65, rows], [1, H * 65]]),
                    in_=v_loc[:rows, :])
            nc.gpsimd.collective_compute(
                "AllGather", ALU.bypass,
                replica_groups=[list(range(NCORES))],
                ins=[ag_in[:].opt()], outs=[ag_out[:].opt()])
            for ot in range(6):
                w_t = wgt.tile([P, FT * P], BF, tag="w768")
                nc.sync.dma_start(out=w_t, in_=d["wqk"][l, ot])
                ps = psA.tile([P, T], F32, tag="mm")
                for kt in range(FT):
                    nc.tensor.matmul(ps, w_t[:, kt * P:(kt + 1) * P],
                                     xln[:, kt * T:(kt + 1) * T],
                                     start=(kt == 0), stop=(kt == FT - 1))
                nc.vector.tensor_scalar_add(out=q_sb[:, ot * T:(ot + 1) * T],
                                            in0=ps,
                                            scalar1=cqk_sb[:, ot:ot + 1])

            # gathered K (feature-major) and V (token-major, 65-col heads)
            kg = pers.tile([P, FT, NK], BF, tag="kg")
            for ft in range(FT):
                nc.sync.dma_start(
                    out=kg[:, ft, :],
                    in_=_ap_view(ag_out, ft * P * T,
                                 [[T, P], [AGE, NCORES], [1, T]]))
            v65 = pers.tile([P, NKT, H * 65], BF, tag="v65")
            for kt in range(NKT):
                kr = min(P, NK - kt * P)
                done = 0
                while done < kr:
                    g = kt * P + done
                    r, t0 = g // T, g % T
                    n = min(kr - done, T - t0)
                    nc.sync.dma_start(
                        out=v65[done:done + n, kt, :],
                        in_=_ap_view(ag_out, r * AGE + VOFF + t0 * (H * 65),
                                     [[H * 65, n], [1, H * 65]]))
                    done += n

            # scoresT + exp (per-head: PSUM DVE reads must stay in-bank)
            expT = pers.tile([P, NKT, H, T], BF, tag="expT")
            for kt in range(NKT):
                kr = min(P, NK - kt * P)
                bias_t = tmpp.tile([P, H, T], BF, tag="biasT")
                nc.sync.dma_start(out=bias_t, in_=d["biasT"][l, kt])
                for h in range(H):
                    bp = 64 * (h % 2)
                    ps = psA.tile([P, T], F32, tag="mm")
                    nc.tensor.matmul(
                        ps[:kr, :],
                        kg[bp:bp + FH, h // 2, kt * P:kt * P + kr],
                        q_sb[bp:bp + FH, (h // 2) * T:(h // 2) * T + T],
                        start=True, stop=True)
                    ssb = tmpp.tile([P, T], F32, tag="ssb")
                    nc.vector.tensor_add(out=ssb[:kr, :], in0=ps[:kr, :],
                                         in1=bias_t[:kr, h, :])
                    nc.scalar.activation(out=expT[:kr, kt, h, :],
                                         in_=ssb[:kr, :], func=AF.Exp)

            # attnV; ones-columns emit denominators as PSUM row 64, and the
            # divide is fused into the eviction
            o_sb = work.tile([P, FT * T], BF, tag="o_sb")
            for h in range(H):
                ps = psA.tile([65, T], F32, tag="mm")
                for kt in range(NKT):
                    kr = min(P, NK - kt * P)
                    nc.tensor.matmul(
                        ps,
                        v65[0:kr, kt, h * 65:(h + 1) * 65],
                        expT[0:kr, kt, h, :],
                        start=(kt == 0), stop=(kt == NKT - 1))
                rec = smalls.tile([1, T], F32, tag="rec")
                nc.vector.reciprocal(out=rec, in_=ps[64:65, :])
                rb = smalls.tile([FH, T], F32, tag="recb")
                nc.gpsimd.partition_broadcast(rb, rec)
                nc.vector.tensor_mul(
                    out=o_sb[64 * (h % 2):64 * (h % 2) + FH,
                             (h // 2) * T:(h // 2 + 1) * T],
                    in0=ps[0:64, :], in1=rb)

            # proj + residual (scale_attn folded into wproj)
            for ot in range(FT):
                w_t = wgt.tile([P, FT * P], BF, tag="w768")
                nc.sync.dma_start(out=w_t, in_=d["wproj"][l, ot])
                ps = psA.tile([P, T], F32, tag="mm")
                for kt in range(FT):
                    nc.tensor.matmul(ps, w_t[:, kt * P:(kt + 1) * P],
                                     o_sb[:, kt * T:(kt + 1) * T],
                                     start=(kt == 0), stop=(kt == FT - 1))
                nc.vector.tensor_add(out=x[:, ot * T:(ot + 1) * T],
                                     in0=x[:, ot * T:(ot + 1) * T], in1=ps)

            # LN2 + MLP
            xln2 = _emit_layernorm(tc, x, ones, eps_t, work, smalls, psA, "ln2")
            m_sb = work.tile([P, MT * T], BF, tag="m_sb")
            for ot in range(MT):
                w_t = wgt.tile([P, FT * P], BF, tag="w768")
                nc.sync.dma_start(out=w_t, in_=d["wfc1"][l, ot])
                ps = psA.tile([P, T], F32, tag="mm")
                for kt in range(FT):
                    nc.tensor.matmul(ps, w_t[:, kt * P:(kt + 1) * P],
                                     xln2[:, kt * T:(kt + 1) * T],
                                     start=(kt == 0), stop=(kt == FT - 1))
                nc.scalar.activation(out=m_sb[:, ot * T:(ot + 1) * T], in_=ps,
                                     func=AF.Gelu,
                                     bias=cfc1_sb[:, ot:ot + 1], scale=1.0)
            for ot in range(FT):
                w_t = wgt2.tile([P, MT * P], BF, tag="w3072")
                nc.sync.dma_start(out=w_t, in_=d["wfc2"][l, ot])
                ps = psA.tile([P, T], F32, tag="mm")
                for kt in range(MT):
                    nc.tensor.matmul(ps, w_t[:, kt * P:(kt + 1) * P],
                                     m_sb[:, kt * T:(kt + 1) * T],
                                     start=(kt == 0), stop=(kt == MT - 1))
                nc.vector.tensor_add(out=x[:, ot * T:(ot + 1) * T],
                                     in0=x[:, ot * T:(ot + 1) * T], in1=ps)

            if l in stage_ends:
                nc.sync.dma_start(out=outs_d[si], in_=x)
                si += 1
        if si == 0:
            nc.sync.dma_start(out=outs_d[0], in_=x)


# ----------------------------------------------------------- host packing --
def _fm_pack(x_tok_feat):
    xr = x_tok_feat.reshape(T, FT, P).transpose(2, 1, 0)
    return np.ascontiguousarray(xr.reshape(P, FT * T))


def _fm_unpack(Xp):
    return np.ascontiguousarray(
        Xp.reshape(P, FT, T).transpose(2, 1, 0).reshape(T, F))


def pack_inputs(inputs):
    f32 = np.float32
    patch = np.asarray(inputs["patch_tokens"], f32)[0]
    cls = np.asarray(inputs["cls_token"], f32)[0]
    x_pad = np.zeros((NK, F), f32)
    x_pad[0] = cls[0]
    x_pad[1:N] = patch

    ln1_w = np.asarray(inputs["ln1_w"], f32)
    ln1_b = np.asarray(inputs["ln1_b"], f32)
    ln2_w = np.asarray(inputs["ln2_w"], f32)
    ln2_b = np.asarray(inputs["ln2_b"], f32)
    qkv_w = np.asarray(inputs["qkv_w"], f32)
    q_bias = np.asarray(inputs["q_bias"], f32).reshape(L, F)
    v_bias = np.asarray(inputs["v_bias"], f32).reshape(L, F)
    proj_w = np.asarray(inputs["proj_w"], f32)
    proj_b = np.asarray(inputs["proj_b"], f32)
    sc_at = np.asarray(inputs["scale_attn"], f32)
    sc_ml = np.asarray(inputs["scale_mlp"], f32)
    fc1_w = np.asarray(inputs["fc1_w"], f32)
    fc1_b = np.asarray(inputs["fc1_b"], f32)
    fc2_w = np.asarray(inputs["fc2_w"], f32)
    fc2_b = np.asarray(inputs["fc2_b"], f32)
    table = np.asarray(inputs["relpos_table"], f32)
    idx = np.asarray(inputs["relpos_index"])

    wq = qkv_w[:, :F, :] * ln1_w[:, None, :] * QK_SCALE
    wk = qkv_w[:, F:2 * F, :] * ln1_w[:, None, :]
    wqk = np.concatenate([wq, wk], axis=1)
    cq = np.einsum("lof,lf->lo", qkv_w[:, :F, :], ln1_b) * QK_SCALE \
        + q_bias * QK_SCALE
    ck = np.einsum("lof,lf->lo", qkv_w[:, F:2 * F, :], ln1_b)
    cqk = np.concatenate([cq, ck], axis=1)

    wv = qkv_w[:, 2 * F:, :] * ln1_w[:, None, :]
    cv = np.einsum("lof,lf->lo", qkv_w[:, 2 * F:, :], ln1_b) + v_bias
    wp = proj_w * sc_at[:, :, None]
    w1 = fc1_w * ln2_w[:, None, :]
    c1 = np.einsum("lof,lf->lo", fc1_w, ln2_b) + fc1_b
    w2 = fc2_w * sc_ml[:, :, None]

    # proj_b / fc2_b enter as extra residual constants; fold them into cqk-
    # style adds only if nonzero (they are zero in this model; assert so).
    assert np.abs(sc_at * proj_b).max() == 0.0
    assert np.abs(sc_ml * fc2_b).max() == 0.0

    def lhsT_pack(w, n_ot, n_k):
        Lh, OF, KF = w.shape
        r = w.reshape(Lh, n_ot, P, n_k, P)
        r = r.transpose(0, 1, 4, 3, 2)
        return np.ascontiguousarray(r).astype(BF16NP)

    wqk_t = lhsT_pack(wqk, 12, FT)
    wp_t = lhsT_pack(wp, FT, FT)
    w1_t = lhsT_pack(w1, MT, FT)
    w2_t = lhsT_pack(w2, FT, MT)

    wv_r = np.zeros((L, FT, P, H * 65), f32)
    wvr = wv.reshape(L, H, FH, FT, P)
    for h in range(H):
        blk = wvr[:, h].transpose(0, 2, 3, 1)
        wv_r[:, :, :, h * 65:h * 65 + FH] = blk
    wv_r = wv_r.astype(BF16NP)

    cv65 = np.zeros((L, 1, H * 65), f32)
    cvr = cv.reshape(L, H, FH)
    for h in range(H):
        cv65[:, 0, h * 65:h * 65 + FH] = cvr[:, h]
        cv65[:, 0, h * 65 + FH] = 1.0

    cqk_p = np.ascontiguousarray(cqk.reshape(L, 12, P).transpose(0, 2, 1))
    c1_p = np.ascontiguousarray(c1.reshape(L, MT, P).transpose(0, 2, 1))

    bias_full = np.empty((L, H, N, N), f32)
    for l in range(L):
        bias_full[l] = table[l][idx].transpose(2, 0, 1)
    biasT_pad = np.zeros((L, H, NK, NK), f32)
    biasT_pad[:, :, :N, :N] = bias_full
    biasT_pad[:, :, :, N:] = -30000.0
    del bias_full

    in_maps = []
    for c in range(NCORES):
        sl = slice(c * T, (c + 1) * T)
        x0 = _fm_pack(x_pad[sl])
        bt = np.zeros((L, NKT, P, H, T), f32)
        for kt in range(NKT):
            kr = min(P, NK - kt * P)
            blk = biasT_pad[:, :, sl, kt * P:kt * P + kr]
            bt[:, kt, :kr] = blk.transpose(0, 3, 1, 2)
        in_maps.append(dict(
            x0=x0.astype(np.float32),
            wqk=wqk_t, cqk=cqk_p.astype(np.float32),
            wv=wv_r, cv=cv65.astype(np.float32),
            wproj=wp_t,
            wfc1=w1_t, cfc1=c1_p.astype(np.float32),
            wfc2=w2_t,
            biasT=bt.astype(BF16NP),
        ))
    return in_maps


def unpack_outputs(results):
    stages = []
    for s in range(4):
        full = np.zeros((NK, F), np.float32)
        for c in range(NCORES):
            full[c * T:(c + 1) * T] = _fm_unpack(results[c]["outs"][s])
        stages.append(np.ascontiguousarray(full[:N][None]))
    return tuple(stages)


# --------------------------------------------------------------- entrypoint --
_NC_CACHE = {}


def get_nc():
    if "nc" not in _NC_CACHE:
        _NC_CACHE["nc"] = build_nc()
    return _NC_CACHE["nc"]


LAST_RUN_NS = None


def kernel(**inputs):
    import time
    global LAST_RUN_NS
    nc = get_nc()
    in_maps = pack_inputs(inputs)
    t0 = time.perf_counter()
    res = bass_utils.run_bass_kernel_spmd(nc, in_maps,
                                          core_ids=list(range(NCORES)))
    LAST_RUN_NS = int((time.perf_counter() - t0) * 1e9)
    return unpack_outputs(res.results)
